# revision 74
# baseline (speedup 1.0000x reference)
"""Trainium2 Bass kernel for KeyeSiglip attention (8192 packed tokens, 8 equal
segments, 16 heads x 72 dim, fused QKV + RoPE + block-diagonal softmax attention
+ output projection).

Sharding: data-parallel over the 8 packed sequences -- one segment per
NeuronCore. Each core runs the full pipeline for its 1024 tokens; outputs are
disjoint row blocks, so no collectives are needed.

Device-side layout (per core, L=1024 tokens):
  P1  qkT[c, t] = (Wqk^T X^T)         18 chunk tiles [128, 1024] bf16
  P2  v_aug[t, c'] = X Wv_aug         [128, 8, 1168] (73 cols/head: ones+bias
                                      marker col 0 for the softmax denominators)
  per head (software-pipelined, skewed):
    gather qh/kh [72,1024] from qkT (sync / gpsimd DMA queues, 4-5 heads
    ahead); RoPE in place: rotate-half via a 72x72 signed-permutation matmul
    on the PE (no cross-partition DMA), then 4 DVE ops;
    scoresT = khT qh -> exp (scalar engine, bf16 probs);  [P4(h) issues
    BEFORE P5(h-1) so the in-order PE queue never waits on exp]
    ctxT_aug = v_aug^T probsT (PSUM, M=73); 1/S via reciprocal_approx_fast;
    partition-broadcast on gpsimd; normalize (DVE); scatter into ctxTc.
  D   out[t, :] = ctxT^T Wout (+bout) -> bf16 [1024, 1152]

Why this shape: the PE is the bottleneck (~600K matmul columns at ~0.5ns/col)
and the chip duty-cycles the PE to ~50% after ~110us of sustained activity
(HAM "activity_1" limiter), so the kernel keeps the PE queue stall-free:
dense projections run first at full rate, attention pipelines rope/exp/norm
across Act+DVE+GpSimd underneath the clamped PE, and the output projection
runs full-rate at the end. Measured ~390us NTFF exec time per core (vs
~753us for the unpipelined int8-I/O version).

Runner: compiles the PJRT executable once, keeps weights device-resident
keyed by a content hash, and ships x / out as bf16.
"""

import zlib
import numpy as np
import ml_dtypes
from contextlib import ExitStack

import jax
import jax.numpy as jnp
from jax.sharding import Mesh, PartitionSpec as P, NamedSharding
from jax.experimental.shard_map import shard_map

import concourse.bass as bass
import concourse.tile as tile
from concourse import bacc, mybir
from concourse.bass2jax import install_neuronx_cc_hook, _bass_exec_p

S_TOT = 8192
H = 1152
NH = 16
HD = 72
NSEG = 8
L = S_TOT // NSEG            # 1024 tokens per core
SCALE = float(HD) ** -0.5
HALF = HD // 2               # 36
DAUG = HD + 1                # 73 (ones column prepended to v for softmax sums)
VW = NH * DAUG               # 1168
NCH_H = H // 128             # 9   hidden-dim chunks
NCH_QK = 2 * H // 128        # 18  q+k channel chunks
BF = mybir.dt.bfloat16
F32 = mybir.dt.float32
BF_NP = ml_dtypes.bfloat16

_STATE_CACHE = {}


def _head_pieces(h):
    """Contiguous (dst_d0, chunk_j, part_p0, n) pieces mapping head-h channels
    [72h, 72h+72) from 128-row chunk layout to a [72, L] per-head tile."""
    pieces = []
    d = 0
    while d < HD:
        c = HD * h + d
        j, p = c // 128, c % 128
        n = min(HD - d, 128 - p)
        pieces.append((d, j, p, n))
        d += n
    return pieces


def build_program(key):
    has_bqk, has_bout = key
    nc = bacc.Bacc("TRN2", target_bir_lowering=False, debug=False,
                   enable_asserts=False)

    xT = nc.dram_tensor("xT", [H, L], BF, kind="ExternalInput").ap()
    wqk = nc.dram_tensor("wqk", [H, 2 * H], BF, kind="ExternalInput").ap()
    wv = nc.dram_tensor("wv", [H, VW], BF, kind="ExternalInput").ap()
    wout = nc.dram_tensor("wout", [H, H], BF, kind="ExternalInput").ap()
    cosT = nc.dram_tensor("cosT", [HD, L], BF, kind="ExternalInput").ap()
    sinT = nc.dram_tensor("sinT", [HD, L], BF, kind="ExternalInput").ap()
    rmat = nc.dram_tensor("rmat", [HD, HD], BF, kind="ExternalInput").ap()
    evec = nc.dram_tensor("evec", [1, VW], BF, kind="ExternalInput").ap()
    bqk = nc.dram_tensor("bqk", [128, NCH_QK], F32, kind="ExternalInput").ap()
    bout = None
    if has_bout:
        bout = nc.dram_tensor("bout", [1, H], BF, kind="ExternalInput").ap()
    out = nc.dram_tensor("out", [L, H], BF, kind="ExternalOutput").ap()

    Ident = mybir.ActivationFunctionType.Identity
    Exp = mybir.ActivationFunctionType.Exp

    with tile.TileContext(nc) as tc, ExitStack() as top:
        # ---- persistent pools (bottom of allocation stack) ----
        persist = top.enter_context(tc.tile_pool(name="persist", bufs=1))
        qkt_pool = top.enter_context(tc.tile_pool(name="qkt", bufs=1))
        ost_pool = top.enter_context(tc.tile_pool(name="ost", bufs=2))
        psum = top.enter_context(tc.tile_pool(name="psum", bufs=4, space="PSUM"))
        # two-bank PSUM tiles for P4: one wide exp drains both q-halves,
        # halving Act instruction + semaphore count
        psum2 = top.enter_context(tc.tile_pool(name="psum2", bufs=2, space="PSUM"))

        v_sb = persist.tile([128, NSEG, VW], BF, name="v_sb", tag="v_sb")
        ctxTc = persist.tile([128, NCH_H, L], BF, name="ctxTc", tag="ctxTc")
        wout_sb = persist.tile([128, NCH_H, H], BF, name="wout_sb", tag="wout_sb")
        cos_sb = persist.tile([HD, L], BF, name="cos_sb", tag="cos_sb")
        sin_sb = persist.tile([HD, L], BF, name="sin_sb", tag="sin_sb")
        rmat_sb = persist.tile([HD, HD], BF, name="rmat_sb", tag="rmat_sb")
        ones_sb = persist.tile([1, 128], BF, name="ones_sb", tag="ones_sb")
        ones73 = persist.tile([1, DAUG], mybir.dt.float16, name="ones73", tag="ones73")
        evec_sb = persist.tile([1, VW], BF, name="evec_sb", tag="evec_sb")
        bqk_sb = persist.tile([128, NCH_QK], F32, name="bqk_sb", tag="bqk_sb")
        bout_sb = persist.tile([1, H], BF, name="bout_sb", tag="bout_sb") if has_bout else None

        nc.vector.memset(ones_sb[:, :], 1.0)
        nc.vector.memset(ones73[:, :], 1.0)

        # qkT chunk tiles [128, L] x 18 (q channels then k channels)
        qkT = [qkt_pool.tile([128, L], BF, name=f"qkT{j}", tag=f"qkT{j}")
               for j in range(NCH_QK)]

        # ---- phase A: projections ----
        # Two nested pools: wqk's 40.5KB/partition frees right after P1, and
        # the per-head gather/rope pools allocate into that space — so rope
        # overlaps P2 instead of waiting for the whole projection phase.
        # small early per-head pools live alongside projA (heads 0-1 rope
        # during P2); big ones open after projA's space frees. Pool stack
        # stays LIFO: wqkp releases first (after P1), then projA.
        hp_e = tc.alloc_tile_pool(name="heads_e", bufs=2)
        swp_e = tc.alloc_tile_pool(name="swp_e", bufs=2)
        pa = tc.alloc_tile_pool(name="projA", bufs=1)
        xt_sb = pa.tile([128, NCH_H, L], BF, name="xt_sb", tag="xt_sb")
        wv_sb = pa.tile([128, NCH_H, VW], BF, name="wv_sb", tag="wv_sb")
        xv = xT.rearrange("(j p) t -> p j t", p=128)
        cc_order = [x for pair in zip(range(NCH_H), range(NCH_H, NCH_QK))
                    for x in pair]
        with tc.tile_pool(name="wqkp", bufs=1) as wqkp:
            wqk_sb = wqkp.tile([128, NCH_H, 2 * H], BF, name="wqk_sb", tag="wqk_sb")

            def wqk_load(eng, cc):
                eng.dma_start(
                    out=wqk_sb[:, :, cc * 128:(cc + 1) * 128],
                    in_=wqk[:, cc * 128:(cc + 1) * 128].rearrange(
                        "(j p) c -> p j c", p=128))

            # lead-in: per-chunk x DMAs (fine-grained semaphores, so P1's
            # first accumulation chain starts on chunk arrival) across all
            # three queues; wqk cc0 front-loaded so the chain has weights
            wqk_load(nc.scalar, cc_order[0])
            for j in range(4):
                nc.sync.dma_start(out=xt_sb[:, j, :], in_=xv[:, j, :])
            for j in range(4, 8):
                nc.scalar.dma_start(out=xt_sb[:, j, :], in_=xv[:, j, :])
            nc.gpsimd.dma_start(out=xt_sb[:, 8, :], in_=xv[:, 8, :])
            # remaining weight chunks: q chunks on scalar, k chunks on sync
            for cc in cc_order[1:]:
                wqk_load(nc.sync if cc >= NCH_H else nc.scalar, cc)
            nc.gpsimd.dma_start(out=wv_sb[:, :, :],
                                in_=wv.rearrange("(j p) c -> p j c", p=128))
            nc.gpsimd.dma_start(out=cos_sb[:, :], in_=cosT)
            nc.gpsimd.dma_start(out=sin_sb[:, :], in_=sinT)
            nc.gpsimd.dma_start(out=rmat_sb[:, :], in_=rmat)
            nc.gpsimd.dma_start(out=evec_sb[:, :], in_=evec)
            nc.gpsimd.dma_start(out=bqk_sb[:, :], in_=bqk)
            if has_bout:
                nc.gpsimd.dma_start(out=bout_sb[:, :], in_=bout)

            # P1: qkT[c, t] = sum_h Wqk[h, c] * X[t, h]   (c-chunk major)
            # q,k interleaved so per-head gathers unlock as early as possible;
            # both t-halves share a two-bank PSUM tile -> one wide drain
            for cc in cc_order:
                ps2 = psum2.tile([128, 2 * 512], F32, name="ps2", tag="ps2")
                for tt in range(2):
                    for hh in range(NCH_H):
                        nc.tensor.matmul(
                            ps2[:, tt * 512:(tt + 1) * 512],
                            lhsT=wqk_sb[:, hh, cc * 128:(cc + 1) * 128],
                            rhs=xt_sb[:, hh, tt * 512:(tt + 1) * 512],
                            start=(hh == 0), stop=(hh == NCH_H - 1))
                if has_bqk:
                    nc.scalar.activation(
                        qkT[cc][:, :], ps2[:, :],
                        Ident, bias=bqk_sb[:, cc:cc + 1])
                else:
                    nc.vector.tensor_copy(qkT[cc][:, :], ps2[:, :])

        # early load of wout (overlaps attention)
        nc.sync.dma_start(out=wout_sb[:, :, :],
                          in_=wout.rearrange("(j p) o -> p j o", p=128))

        def stage_gather(h, hp):
            """DMA-gather pre-rope qh/kh for head h; returns (qh, kh)"""
            qh = hp.tile([HD, L], BF, name="qh", tag="qh")
            kh = hp.tile([HD, L], BF, name="kh", tag="kh")
            for dst, base, geng in ((qh, 0, nc.sync), (kh, NCH_H, nc.gpsimd)):
                for (d0, j, p0, n) in _head_pieces(h):
                    geng.dma_start(out=dst[d0:d0 + n, :],
                                   in_=qkT[base + j][p0:p0 + n, :])
            return qh, kh

        def stage_rope(qh, kh, swp):
            """in-place rope: t = t*cos + (R t)*sin, rotate-half via a 72x72
            signed-permutation matmul on the PE (no cross-partition DMA)"""
            for t_ in (qh, kh):
                tmp = swp.tile([HD, L], BF, name="swtmp", tag="swtmp")
                for qt in range(2):
                    psr = psum.tile([128, 512], F32, name="ps", tag="ps")
                    nc.tensor.matmul(psr[0:HD, :], lhsT=rmat_sb[:, :],
                                     rhs=t_[:, qt * 512:(qt + 1) * 512],
                                     start=True, stop=True)
                    nc.vector.tensor_mul(
                        tmp[:, qt * 512:(qt + 1) * 512], psr[0:HD, :],
                        sin_sb[:, qt * 512:(qt + 1) * 512])
                # in-place cos-mul; Tile orders it after the rotate matmuls
                # read the pre-rope rows
                nc.vector.tensor_mul(t_[:, :], t_[:, :], cos_sb[:, :])
                nc.vector.tensor_add(t_[:, :], t_[:, :], tmp[:, :])
            return qh, kh

        # P2: v[t, c'] = sum_h X[t, h] * Wv_aug[h, c']  (+ marker/bias row);
        # the two 512-wide slices share a two-bank PSUM tile -> one wide drain
        for tt in range(NSEG):
            ps2 = psum2.tile([128, 2 * 512], F32, name="ps2", tag="ps2")
            ps3 = psum.tile([128, 512], F32, name="ps", tag="ps")
            for hh in range(NCH_H):
                for o0 in (0, 512):
                    nc.tensor.matmul(
                        ps2[:, o0:o0 + 512],
                        lhsT=xt_sb[:, hh, tt * 128:(tt + 1) * 128],
                        rhs=wv_sb[:, hh, o0:o0 + 512],
                        start=(hh == 0), stop=False)
                nc.tensor.matmul(
                    ps3[:, :VW - 1024],
                    lhsT=xt_sb[:, hh, tt * 128:(tt + 1) * 128],
                    rhs=wv_sb[:, hh, 1024:VW],
                    start=(hh == 0), stop=False)
            for o0 in (0, 512):
                nc.tensor.matmul(
                    ps2[:, o0:o0 + 512],
                    lhsT=ones_sb[:, :],
                    rhs=evec_sb[:, o0:o0 + 512],
                    start=False, stop=True)
            nc.tensor.matmul(
                ps3[:, :VW - 1024],
                lhsT=ones_sb[:, :],
                rhs=evec_sb[:, 1024:VW],
                start=False, stop=True)
            nc.vector.tensor_copy(v_sb[:, tt, 0:1024], ps2[:, :])
            nc.vector.tensor_copy(v_sb[:, tt, 1024:VW], ps3[:, :VW - 1024])

        # gathers for the first heads (DMA) + head-0 rope overlap P2
        gathered, roped = {}, {}
        GATHER_AHEAD = 5
        for h in range(2):
            gathered[h] = stage_gather(h, hp_e)
        roped[0] = stage_rope(*gathered.pop(0), swp_e)

        pa.release()
        hp2 = tc.alloc_tile_pool(name="heads2", bufs=6)
        swp2 = tc.alloc_tile_pool(name="swp2", bufs=3)
        for h in range(2, GATHER_AHEAD + 2):
            gathered[h] = stage_gather(h, hp2)

        # ---- phase B+C: per-head attention ----
        # Software-pipelined with a 1-head skew: P4(h+1) is issued BEFORE
        # P5(h), so the in-order PE queue works on the next head's score
        # matmuls while the Act engine exponentiates the current head's —
        # instead of stalling behind P5(h)'s wait on exp(h).
        with tc.tile_pool(name="probs_p", bufs=16) as pp, \
             tc.tile_pool(name="ctx_p", bufs=2) as cp, \
             tc.tile_pool(name="norm_p", bufs=2) as npp:
            probs_of = {}

            def stage_front(h):
                """scores + exp for head h"""
                qh, kh = roped.pop(h)
                # P4: probsT[k, q] = exp(SCALE * k.q), 8 k-tiles
                probs = [pp.tile([128, L], BF, name="probs", tag="probs")
                         for _ in range(NSEG)]
                probs_of[h] = probs
                for kt in range(NSEG):
                    ps2 = psum2.tile([128, 2 * 512], F32, name="ps2", tag="ps2")
                    for qt in range(2):
                        nc.tensor.matmul(
                            ps2[:, qt * 512:(qt + 1) * 512],
                            lhsT=kh[:, kt * 128:(kt + 1) * 128],
                            rhs=qh[:, qt * 512:(qt + 1) * 512],
                            start=True, stop=True)
                    nc.scalar.activation(probs[kt][:, :], ps2[:, :],
                                         Exp, scale=SCALE)

            def stage_back(h):
                """AV + normalize + scatter for head h. No SBUF staging of
                the raw context: the reciprocal reads the S row straight from
                PSUM and the normalize multiply drains PSUM -> bf16 SBUF."""
                probs = probs_of.pop(h)
                pss = []
                for qt in range(2):
                    ps = psum.tile([128, 512], F32, name="ps", tag="ps")
                    for kt in range(NSEG):
                        nc.tensor.matmul(
                            ps[0:DAUG, :],
                            lhsT=v_sb[:, kt, h * DAUG:(h + 1) * DAUG],
                            rhs=probs[kt][:, qt * 512:(qt + 1) * 512],
                            start=(kt == 0), stop=(kt == NSEG - 1))
                    pss.append(ps)

                # row 0 of the PSUM context is S; rows 1..72 are ctx dims
                rr32 = npp.tile([1, L], F32, name="rr32", tag="rr32")
                for qt in range(2):
                    nc.vector.reciprocal_approx_fast(
                        rr32[:, qt * 512:(qt + 1) * 512], pss[qt][0:1, :])
                rr16 = npp.tile([1, L], BF, name="rr16", tag="rr16")
                with nc.allow_low_precision(reason="softmax recip row"):
                    nc.vector.tensor_copy(rr16[:, :], rr32[:, :])
                rb = npp.tile([DAUG, L], BF, name="rb", tag="rb")
                nc.gpsimd.partition_broadcast(rb[:, :], rr16[:, :])
                ctxn = npp.tile([DAUG, L], BF, name="ctxn", tag="ctxn")
                for qt in range(2):
                    nc.vector.tensor_mul(
                        ctxn[:, qt * 512:(qt + 1) * 512], pss[qt][0:DAUG, :],
                        rb[:, qt * 512:(qt + 1) * 512])
                for (d0, j, p0, n) in _head_pieces(h):
                    nc.sync.dma_start(out=ctxTc[p0:p0 + n, j, :],
                                      in_=ctxn[1 + d0:1 + d0 + n, :])

            for h in range(NH + 1):
                if h < NH:
                    # PE program order per slot: rotate(h+1), P4(h), P5(h-1)
                    if h + 1 < NH:
                        roped[h + 1] = stage_rope(*gathered.pop(h + 1), swp2)
                    if h + GATHER_AHEAD + 2 < NH:
                        gathered[h + GATHER_AHEAD + 2] = stage_gather(
                            h + GATHER_AHEAD + 2, hp2)
                    stage_front(h)
                if h >= 1:
                    stage_back(h - 1)
        swp2.release()
        hp2.release()
        swp_e.release()
        hp_e.release()

        # ---- phase D: output projection ----
        oslices = [(0, 384), (384, 384), (768, 384)]
        for tt in range(NSEG):
            pso = [psum.tile([128, 512], F32, name="ps", tag="ps") for _ in oslices]
            for cc in range(NCH_H):
                for oi, (o0, w) in enumerate(oslices):
                    nc.tensor.matmul(
                        pso[oi][:, :w],
                        lhsT=ctxTc[:, cc, tt * 128:(tt + 1) * 128],
                        rhs=wout_sb[:, cc, o0:o0 + w],
                        start=(cc == 0), stop=(cc == NCH_H - 1 and not has_bout))
            if has_bout:
                for oi, (o0, w) in enumerate(oslices):
                    nc.tensor.matmul(
                        pso[oi][:, :w],
                        lhsT=ones_sb[:, :],
                        rhs=bout_sb[:, o0:o0 + w],
                        start=False, stop=True)
            ost = ost_pool.tile([128, H], BF, name="ost", tag="ost")
            for oi, (o0, w) in enumerate(oslices):
                # Act engine is idle in phase D; drain PSUM there
                nc.scalar.activation(ost[:, o0:o0 + w], pso[oi][:, :w], Ident)
            nc.sync.dma_start(out=out[tt * 128:(tt + 1) * 128, :],
                              in_=ost[:, :])

    nc.compile()
    return nc


# ---------------------------------------------------------------------------
# Runner: cached PJRT executable + device-resident weights.
# ---------------------------------------------------------------------------

class _State:
    __slots__ = ("nc", "mesh", "sh", "in_names", "out_names", "out_avals",
                 "bass_fn", "mkz", "resident", "weights_sig", "next_zeros")


def _build_state(key):
    """Compile the Bass program and build the cached jitted callable."""
    install_neuronx_cc_hook()
    nc = build_program(key)
    assert nc.dbg_addr is None and not nc.dbg_callbacks

    st = _State()
    st.nc = nc
    devs = jax.devices()[:NSEG]
    assert len(devs) == NSEG, f"need {NSEG} devices, have {len(jax.devices())}"
    st.mesh = Mesh(np.asarray(devs), ("core",))
    st.sh = NamedSharding(st.mesh, P("core"))

    part_name = nc.partition_id_tensor.name if nc.partition_id_tensor else None
    in_names, out_names, out_avals = [], [], []
    for alloc in nc.m.functions[0].allocations:
        if not isinstance(alloc, mybir.MemoryLocationSet):
            continue
        name = alloc.memorylocations[0].name
        if alloc.kind == "ExternalInput":
            if name != part_name:
                in_names.append(name)
        elif alloc.kind == "ExternalOutput":
            out_names.append(name)
            out_avals.append(jax.core.ShapedArray(
                tuple(alloc.tensor_shape), mybir.dt.np(alloc.dtype)))
    st.in_names = in_names
    st.out_names = out_names
    st.out_avals = out_avals
    all_names = tuple(in_names) + tuple(out_names)
    if part_name is not None:
        all_names = all_names + (part_name,)

    def _body(*args):
        operands = list(args)
        if part_name is not None:
            from concourse.bass2jax import partition_id_tensor
            operands.append(partition_id_tensor())
        return tuple(_bass_exec_p.bind(
            *operands,
            out_avals=tuple(out_avals),
            in_names=all_names,
            out_names=tuple(out_names),
            lowering_input_output_aliases=(),
            sim_require_finite=True,
            sim_require_nnan=True,
            nc=nc,
        ))

    # No donation: the kernel writes every element of every output, so one
    # persistent output-binding buffer set serves all calls.
    n_in = len(in_names)
    st.bass_fn = jax.jit(
        shard_map(_body, mesh=st.mesh,
                  in_specs=(P("core"),) * (n_in + len(out_names)),
                  out_specs=(P("core"),) * len(out_names)),
        keep_unused=True)

    zshapes = [(NSEG * a.shape[0], *a.shape[1:]) for a in out_avals]
    zdtypes = [a.dtype for a in out_avals]
    st.mkz = jax.jit(
        lambda: tuple(jnp.zeros(s, d) for s, d in zip(zshapes, zdtypes)),
        out_shardings=tuple(st.sh for _ in out_avals))

    st.resident = None
    st.weights_sig = None
    st.next_zeros = None
    return st


def _get_state(key):
    if key not in _STATE_CACHE:
        _STATE_CACHE[key] = _build_state(key)
    return _STATE_CACHE[key]


def _weights_sig(arrs):
    sig = 0
    for a in arrs:
        a = np.ascontiguousarray(a)
        sig = zlib.adler32(a.view(np.uint8).reshape(-1), sig)
        sig = zlib.adler32(repr((a.shape, a.dtype.str)).encode(), sig)
    return sig


def _fast_bf16(x):
    """float32 ndarray -> bfloat16 via round-to-nearest-even bit twiddling
    (ml_dtypes .astype is ~10x slower)."""
    x = np.ascontiguousarray(x, np.float32)
    u = x.view(np.uint32)
    rounded = (u + 0x7FFF + ((u >> 16) & 1)) >> 16
    return rounded.astype(np.uint16).view(BF_NP).reshape(x.shape)


def prep_weights(cos, sin, Wqkv, bqkv, Wout, bout):
    """Host-side layout prep for the non-activation inputs. Returns
    (key, sig, dict name -> global [NSEG*rows, cols] numpy array)."""
    cos = np.asarray(cos, np.float32)
    sin = np.asarray(sin, np.float32)
    Wqkv = np.asarray(Wqkv, np.float32)
    bqkv = np.asarray(bqkv, np.float32)
    Wout = np.asarray(Wout, np.float32)
    bout = np.asarray(bout, np.float32)
    sig = _weights_sig((cos, sin, Wqkv, bqkv, Wout, bout))

    wqk_np = _fast_bf16(Wqkv[:, :2 * H])
    wv = Wqkv[:, 2 * H:]
    wv_aug = np.zeros((H, VW), np.float32)
    for h in range(NH):
        wv_aug[:, h * DAUG + 1:h * DAUG + 1 + HD] = wv[:, h * HD:(h + 1) * HD]
    wv_np = _fast_bf16(wv_aug)
    wout_np = _fast_bf16(Wout)

    evec = np.zeros((1, VW), np.float32)
    for h in range(NH):
        evec[0, h * DAUG + 1:h * DAUG + 1 + HD] = bqkv[2 * H + h * HD:2 * H + (h + 1) * HD]
        evec[0, h * DAUG] = 1.0
    evec_np = _fast_bf16(evec)
    bqk_np = np.ascontiguousarray(bqkv[:2 * H].reshape(NCH_QK, 128).T).astype(np.float32)
    has_bqk = bool(np.any(bqkv[:2 * H]))
    has_bout = bool(np.any(bout))
    key = (has_bqk, has_bout)

    # per-core cos/sin slices (stacked); the rotate-half sign lives in rmat
    cosT = np.stack([cos[s * L:(s + 1) * L].T for s in range(NSEG)])
    sinT = np.stack([sin[s * L:(s + 1) * L].T for s in range(NSEG)])

    # signed rotate-half permutation: (R t)[d'] = -t[d'+36] (d'<36),
    # +t[d'-36] (d'>=36); lhsT convention: out[d'] = sum_d rmat[d, d'] t[d]
    rmat = np.zeros((HD, HD), np.float32)
    for dp in range(HALF):
        rmat[dp + HALF, dp] = -1.0
    for dp in range(HALF, HD):
        rmat[dp - HALF, dp] = 1.0

    g = {
        "rmat": np.broadcast_to(_fast_bf16(rmat), (NSEG, HD, HD)).reshape(
            NSEG * HD, HD),
        "wqk": np.broadcast_to(wqk_np, (NSEG, H, 2 * H)).reshape(NSEG * H, 2 * H),
        "wv": np.broadcast_to(wv_np, (NSEG, H, VW)).reshape(NSEG * H, VW),
        "wout": np.broadcast_to(wout_np, (NSEG, H, H)).reshape(NSEG * H, H),
        "cosT": _fast_bf16(cosT).reshape(NSEG * HD, L),
        "sinT": _fast_bf16(sinT).reshape(NSEG * HD, L),
        "evec": np.broadcast_to(evec_np, (NSEG, 1, VW)).reshape(NSEG, VW),
        "bqk": np.broadcast_to(bqk_np, (NSEG, 128, NCH_QK)).reshape(NSEG * 128, NCH_QK),
    }
    if has_bout:
        g["bout"] = np.broadcast_to(_fast_bf16(bout.reshape(1, H)),
                                    (NSEG, 1, H)).reshape(NSEG, H)
    return key, sig, g


def prep_x(hidden_states):
    """Host: [1, S, H] f32 -> per-core-stacked transposed bf16 [NSEG*H, L]."""
    hs = np.asarray(hidden_states, np.float32).reshape(NSEG, L, H)
    xT = np.ascontiguousarray(hs.transpose(0, 2, 1))           # [NSEG, H, L]
    return _fast_bf16(xT).reshape(NSEG * H, L)


def ensure_weights(key, sig, g):
    """Upload weight/constant tensors to device HBM if not already resident."""
    st = _get_state(key)
    if st.weights_sig != sig:
        st.resident = {n: jax.device_put(a, st.sh) for n, a in g.items()}
        for a in st.resident.values():
            a.block_until_ready()
        st.weights_sig = sig
    if st.next_zeros is None:
        st.next_zeros = st.mkz()   # persistent output-binding buffers
        jax.block_until_ready(st.next_zeros)
    return st


def run_prepped(st, xT_np):
    """One full device execution: upload x, run, fetch output.
    Returns host f32 [1, S, H]. This is the steady-state per-call path."""
    xdev = jax.device_put(xT_np, st.sh)
    args = [xdev if n == "xT" else st.resident[n] for n in st.in_names]
    outs = st.bass_fn(*args, *st.next_zeros)
    res = np.asarray(jax.device_get(outs[st.out_names.index("out")]),
                     dtype=np.float32)
    return res.reshape(1, S_TOT, H)


def kernel(**inputs):
    x = inputs.pop("hidden_states")
    inputs.pop("cu_seqlens", None)
    key, sig, g = prep_weights(**inputs)
    st = ensure_weights(key, sig, g)
    return run_prepped(st, prep_x(x))
